# revision 10
# baseline (speedup 1.0000x reference)
"""Trainium2 Bass kernel for channel-attention + 2 residual conv blocks.

Data-parallel over batch (8 cores, 1 batch element each). Two SPMD launches:
  A) raw channel Gram G = [x;y]^T-pixel-contraction ([128,128]) from fp8
     pixel-major data via 256 DoubleRow-accumulating matmuls (2 pixels per
     PE cell-row); DMA-bound at ~23us.
  B) fused attention-apply + 4 3x3 convs, streaming the full image in 8-row
     waves (no block halos). Convs use K=128 dup-stacked fp8 DoubleRow
     matmuls: each stage ring keeps a row-shifted duplicate in partitions
     64-127 (filled by SBUF->SBUF DMA), so one matmul covers 4 of the 9
     taps for both row parities -> 3 matmuls per 4-row half-group. conv1 is
     algebraically folded through the attention (W1' = W1 @ Wav) so it
     reads y directly (host-uploaded fp8 dup form) and the t0 stream
     disappears; the boundary bias terms the fold breaks are restored
     exactly with K=1 corrective matmuls into PSUM. Residual paths stay
     exact via bf16 wavt/identity injection matmuls. Output leaves as
     (parity,channel)-major f32 [128,128,256] and the host de-permutes.
Host does only the O(64^2) softmax/fold algebra between launches.
"""
import sys, os
for p in ('/opt/trn_rl_repo', os.path.expanduser('~/.axon_site/_ro/trn_rl_repo')):
    if os.path.isdir(p) and p not in sys.path:
        sys.path.insert(0, p)

import numpy as np
import ml_dtypes
import concourse.bass as bass
import concourse.bacc as bacc
import concourse.tile as tile
from concourse import mybir
from concourse.ap import AP
from concourse.bass_utils import run_bass_kernel_spmd

dt = mybir.dt
F32, BF16, FP8 = dt.float32, dt.bfloat16, dt.float8e4
BF = ml_dtypes.bfloat16
E4 = ml_dtypes.float8_e4m3
AF = mybir.ActivationFunctionType
OP = mybir.AluOpType
DR = mybir.MatmulPerfMode.DoubleRow

D = 64
HW = 65536
H = W_IMG = 256
NCORES = 8
RS = 258          # padded stage row stride (cols -1..256)
RB_S = 128        # stage ring rows (c1 / o3f / c3)
RB_Y8 = 32        # yf8d ring rows
RB_OY = 128       # oy ring rows
NW = 32           # 8-row waves
L2, L3, L4 = 3, 7, 11  # wave lags conv2/3/4
EPS = 1e-12

# DR matmul slot/col positions (sigma, chi) per (m, t): position (sigma, chi)
# at out-base row R covers image rows R+sigma (partitions 0-63) and
# R+sigma+1 (dup partitions 64-127), shifted chi columns. t-strides must
# avoid 1 (hardware rejects); these give strides {2, 2*RS, 2}.
MPOS = [((-1, -1), (-1, +1)),
        ((-1, 0), (+1, 0)),
        ((+1, -1), (+1, +1))]


def _conv_w3(Wtap):
    """Wtap[ky+1][kx+1] = [out, in] f64 -> 3 arrays [128, 2, 128] e4m3."""
    out = []
    for m in range(3):
        Wm = np.zeros((128, 2, 128), np.float64)
        for t in range(2):
            sig, chi = MPOS[m][t]
            for d_ in range(2):
                for rho in range(2):
                    ky = sig + d_ - rho
                    if -1 <= ky <= 1:
                        Wm[d_*64:(d_+1)*64, t, rho*64:(rho+1)*64] = \
                            Wtap[ky+1][chi+1].T
        out.append(Wm.astype(E4))
    return out


def _build_nc_a():
    nc = bacc.Bacc("TRN2", target_bir_lowering=False, debug=False)
    za = nc.dram_tensor("za", [16, 128, 4096], FP8, kind="ExternalInput").ap()
    gout = nc.dram_tensor("gout", [128, 128], F32, kind="ExternalOutput").ap()
    with tile.TileContext(nc) as tc:
        with tc.tile_pool(name="io", bufs=4) as io, \
             tc.tile_pool(name="work", bufs=1) as work, \
             tc.tile_pool(name="gps", bufs=1, space="PSUM") as gps:
            gp = gps.tile([128, 128], F32)
            for t in range(16):
                zt = io.tile([128, 4096], FP8, tag="zt")
                nc.sync.dma_start(out=zt, in_=za[t])
                for j in range(16):
                    s = zt[:, j * 256:(j + 1) * 256].rearrange(
                        "p (t c) -> p t c", t=2)
                    nc.tensor.matmul(gp, s, s,
                                     start=(t == 0 and j == 0),
                                     stop=(t == 15 and j == 15),
                                     perf_mode=DR, skip_group_check=True)
            gs = work.tile([128, 128], F32)
            nc.vector.tensor_copy(out=gs, in_=gp)
            nc.sync.dma_start(out=gout, in_=gs)
    nc.compile()
    return nc


def _mv_ap(stage, rb, R, m):
    """DR moving AP for half-group at out-base row R, matmul variant m.
    Stage ring [128, rb, RS]; plane p0-63 slot q = row q-1, p64-127 slot q
    = row q. Caller guarantees slots q=R..R+4 don't wrap."""
    (s0, c0), (s1, c1) = MPOS[m]
    base = stage[:, 0:1, 0:1]
    q0 = (R + s0 + 1) % rb
    tstride = (s1 - s0) * RS + (c1 - c0)
    off = base.offset + q0 * RS + (c0 + 1)
    return AP(base.tensor, off,
              [[base.ap[0][0], 128], [tstride, 2], [2 * RS, 2], [1, 256]])


def _emit_conv(nc, ps_half, ws, stage, rb, R, first, stop_last=False):
    """One 4-row half-group conv: 3 DR matmuls (wrap-aware) into the psum
    half [128, 512]."""
    if (R + 1) % rb <= rb - 5:  # slots R..R+4 contiguous in the ring
        for m in range(3):
            nc.tensor.matmul(ps_half, ws[m], _mv_ap(stage, rb, R, m),
                             start=(first and m == 0),
                             stop=(stop_last and m == 2),
                             perf_mode=DR, skip_group_check=True)
        return
    # wrap fallback: per-(m, s) pieces; t-pairs that straddle the ring edge
    # become two plain (non-DR) single-plane matmuls
    base = stage[:, 0:1, 0:1]
    pstr = base.ap[0][0]
    plan = []
    for m in range(3):
        (s0, c0), (s1, c1) = MPOS[m]
        for s_i in range(2):
            qa = R + s0 + 1 + 2 * s_i
            qb = R + s1 + 1 + 2 * s_i
            ph = ps_half[:, s_i * 256:(s_i + 1) * 256]
            if (qa % rb) + (qb - qa) == qb % rb:
                off = base.offset + (qa % rb) * RS + (c0 + 1)
                mv = AP(base.tensor, off,
                        [[pstr, 128], [(qb - qa) * RS + (c1 - c0), 2],
                         [1, 256]])
                plan.append((ph, ws[m], mv, True))
            else:
                for t_sel, q in ((0, qa), (1, qb)):
                    chi = MPOS[m][t_sel][1]
                    off = base.offset + (q % rb) * RS + (chi + 1)
                    mv = AP(base.tensor, off, [[pstr, 128], [1, 256]])
                    plan.append((ph, ws[m][:, t_sel, :], mv, False))
    for i, (ph, w_, mv, is_dr) in enumerate(plan):
        nc.tensor.matmul(ph, w_, mv,
                         start=(first and i == 0),
                         stop=(stop_last and i == len(plan) - 1),
                         perf_mode=(DR if is_dr else None),
                         skip_group_check=True)


def _build_nc_b():
    nc = bacc.Bacc("TRN2", target_bir_lowering=False, debug=False)
    yb = nc.dram_tensor("yb", [D, H, W_IMG], BF16, kind="ExternalInput").ap()
    y8 = nc.dram_tensor("y8", [128, 264, RS], FP8, kind="ExternalInput").ap()
    wavt_d = nc.dram_tensor("wavt", [D, D], BF16, kind="ExternalInput").ap()
    ii_d = nc.dram_tensor("ii", [2 * D, D], BF16, kind="ExternalInput").ap()
    corr_d = nc.dram_tensor("corr", [1, 10 * D], BF16, kind="ExternalInput").ap()
    wdr_d = {}
    for c in range(1, 5):
        for m in range(3):
            nm = f"w{c}{m}"
            wdr_d[nm] = nc.dram_tensor(nm, [128, 2, 2 * D], FP8,
                                       kind="ExternalInput").ap()
    bias_d = {nm: nc.dram_tensor(nm, [2 * D, 1], F32, kind="ExternalInput").ap()
              for nm in ('bc1', 'bo3b', 'bn4', 'bc3')}
    out_d = nc.dram_tensor("out", [128, 128, W_IMG], F32,
                           kind="ExternalOutput").ap()

    with tile.TileContext(nc) as tc:
        with tc.tile_pool(name="consts", bufs=1) as consts, \
             tc.tile_pool(name="stg", bufs=1) as stg, \
             tc.tile_pool(name="ots", bufs=2) as ots, \
             tc.tile_pool(name="ps1", bufs=1, space="PSUM") as ps1, \
             tc.tile_pool(name="ps2", bufs=1, space="PSUM") as ps2, \
             tc.tile_pool(name="ps3", bufs=1, space="PSUM") as ps3, \
             tc.tile_pool(name="ps4", bufs=1, space="PSUM") as ps4:
            # first data loads lead everything on the SP queue
            yf8 = stg.tile([128, RB_Y8, RS], FP8)
            for k in range(3):
                nc.sync.dma_start(out=yf8[:, 8 * k:8 * k + 8, :],
                                  in_=y8[:, 8 * k:8 * k + 8, :])
            oy = stg.tile([128, RB_OY, W_IMG], BF16)
            nc.sync.dma_start(out=oy[0:D, 0:8, :], in_=yb[:, 0:8, :])

            # consts on the scalar/gpsimd queues
            wavt = consts.tile([D, D], BF16)
            ii = consts.tile([2 * D, D], BF16)
            corr = consts.tile([1, 10 * D], BF16)
            ones = consts.tile([1, 256], BF16)
            nc.scalar.dma_start(out=wavt, in_=wavt_d)
            nc.scalar.dma_start(out=ii, in_=ii_d)
            nc.scalar.dma_start(out=corr, in_=corr_d)
            nc.vector.memset(ones, 1.0)
            wdr = {}
            for nm, d_ in wdr_d.items():
                t = consts.tile([128, 2, 2 * D], FP8, tag=nm)
                nc.gpsimd.dma_start(out=t, in_=d_)
                wdr[nm] = t
            bias = {}
            for nm, d_ in bias_d.items():
                t = consts.tile([2 * D, 1], F32, tag=nm)
                nc.scalar.dma_start(out=t, in_=d_)
                bias[nm] = t

            c1 = stg.tile([128, RB_S, RS], FP8)
            o3f = stg.tile([128, RB_S, RS], FP8)
            c3 = stg.tile([128, RB_S, RS], FP8)
            # halo cols + virtual row -1 (row-(q-1) plane, slot 0)
            for s in (c1, o3f, c3):
                nc.vector.memset(s[0:D, 0:1, :], 0.0)
                nc.vector.memset(s[:, :, 0:1], 0.0)
                nc.vector.memset(s[:, :, 257:258], 0.0)

            def wr(dst, src, b, relu, eng='a', scale=1.0):
                if eng == 'a':
                    nc.scalar.activation(out=dst, in_=src,
                                         func=(AF.Relu if relu else AF.Identity),
                                         bias=b, scale=scale)
                elif relu:
                    nc.vector.tensor_scalar(out=dst, in0=src, scalar1=b,
                                            scalar2=0.0, op0=OP.add, op1=OP.max)
                else:
                    nc.vector.tensor_scalar_add(out=dst, in0=src, scalar1=b)

            def wr_stage(stage_, psum, G, b, eng, scale=1.0):
                """relu+bias psum [128,1024] -> stage rows G..G+7 (wrap-aware
                ring writes in the row-(q-1) plane, slots rows+1)."""
                for rho in range(2):
                    q0 = (G + rho + 1) % RB_S
                    src = psum[rho * D:(rho + 1) * D, :]
                    e = eng[rho]
                    if q0 + 6 <= RB_S - 1:
                        wr(stage_[0:D, q0:q0 + 7:2, 1:257], src, b, True,
                           e, scale)
                    else:
                        n1 = (RB_S - q0 + 1) // 2
                        pv = psum.rearrange("p (a c) -> p a c", c=256)
                        wr(stage_[0:D, q0:q0 + 2 * n1 - 1:2, 1:257],
                           pv[rho * D:(rho + 1) * D, 0:n1, :],
                           b, True, e, scale)
                        if n1 < 4:
                            q1 = (q0 + 2 * n1) % RB_S
                            wr(stage_[0:D, q1:q1 + 2 * (4 - n1) - 1:2, 1:257],
                               pv[rho * D:(rho + 1) * D, n1:4, :],
                               b, True, e, scale)

            def dup_dma(stage_, k, src_lo):
                """Dup chunk k between the two row planes of a stage ring.
                src_lo True (c1/c3): [64:128] slot q <- [0:64] slot q+1,
                q in [8k, 8k+8). src_lo False (o3f): [0:64] slot q <-
                [64:128] slot q-1, q in [8k+1, 8k+9)."""
                if src_lo:
                    dq0, sq0 = 8 * k, 8 * k + 1
                    dpr, spr = slice(D, 2 * D), slice(0, D)
                else:
                    dq0, sq0 = 8 * k + 1, 8 * k
                    dpr, spr = slice(0, D), slice(D, 2 * D)
                left = 8
                while left:
                    d0, s0 = dq0 % RB_S, sq0 % RB_S
                    run = min(left, RB_S - d0, RB_S - s0)
                    nc.scalar.dma_start(out=stage_[dpr, d0:d0 + run, :],
                                        in_=stage_[spr, s0:s0 + run, :])
                    dq0 += run
                    sq0 += run
                    left -= run

            # ---- wave loop ------------------------------------------------
            for w in range(NW + L4 + 1):
                # stream loads (lead the consumers)
                if w + 3 <= 32:
                    sl = (8 * (w + 3)) % RB_Y8
                    nc.sync.dma_start(out=yf8[:, sl:sl + 8, :],
                                      in_=y8[:, 8 * (w + 3):8 * (w + 3) + 8, :])
                if 1 <= w + 1 < 32:
                    sl = (8 * (w + 1)) % RB_OY
                    nc.sync.dma_start(out=oy[0:D, sl:sl + 8, :],
                                      in_=yb[:, 8 * (w + 1):8 * (w + 1) + 8, :])

                # virtual row 256 (row-q plane, slot 0): zero after each
                # stage's q=128 consumers are done (conv2 at w=L2+16, conv3
                # at L3+16, conv4 at L4+16), before its wave-31 consumer
                if w == L2 + 18:
                    nc.vector.memset(c1[D:2 * D, 0:1, :], 0.0)
                if w == L3 + 18:
                    nc.vector.memset(o3f[D:2 * D, 0:1, :], 0.0)
                if w == L4 + 18:
                    nc.vector.memset(c3[D:2 * D, 0:1, :], 0.0)

                # ---- conv1 (reads yf8d) -> c1
                if w < NW:
                    G = 8 * w
                    p1 = ps1.tile([128, 1024], F32, tag="p1")
                    for h in range(2):
                        _emit_conv(nc, p1[:, h * 512:(h + 1) * 512],
                                   [wdr[f"w1{m}"] for m in range(3)],
                                   yf8, RB_Y8, G + 4 * h, first=True)
                    # exact boundary-bias restoration for the conv1 fold
                    if w == 0:
                        nc.tensor.matmul(p1[0:D, 0:256], corr[:, 256:320],
                                         ones[:, 0:256], start=False,
                                         stop=False, skip_group_check=True)
                        nc.tensor.matmul(p1[0:D, 0:1], corr[:, 384:448],
                                         ones[:, 0:1], start=False, stop=False,
                                         skip_group_check=True)
                        nc.tensor.matmul(p1[0:D, 255:256], corr[:, 448:512],
                                         ones[:, 0:1], start=False, stop=False,
                                         skip_group_check=True)
                    if w == NW - 1:
                        nc.tensor.matmul(p1[D:2 * D, 768:1024],
                                         corr[:, 320:384], ones[:, 0:256],
                                         start=False, stop=False,
                                         skip_group_check=True)
                        nc.tensor.matmul(p1[D:2 * D, 768:769],
                                         corr[:, 512:576], ones[:, 0:1],
                                         start=False, stop=False,
                                         skip_group_check=True)
                        nc.tensor.matmul(p1[D:2 * D, 1023:1024],
                                         corr[:, 576:640], ones[:, 0:1],
                                         start=False, stop=False,
                                         skip_group_check=True)
                    pv = p1.rearrange("p (a c) -> p a c", c=256)
                    for h in range(2):
                        nc.tensor.matmul(pv[:, 2 * h:2 * h + 2, 0:1],
                                         corr[:, 0:128], ones[:, 0:2],
                                         start=False, stop=False,
                                         skip_group_check=True)
                        nc.tensor.matmul(pv[:, 2 * h:2 * h + 2, 255:256],
                                         corr[:, 128:256], ones[:, 0:2],
                                         start=False, stop=(h == 1),
                                         skip_group_check=True)
                    wr_stage(c1, p1, G, bias['bc1'][0:D], ('a', 'a'), 0.25)
                    dup_dma(c1, w, src_lo=True)

                # ---- conv2 (reads c1) + wavt-inj -> o3b in oy[64:128]
                v = w - L2
                if 0 <= v < NW:
                    G = 8 * v
                    p2 = ps2.tile([128, 1024], F32, tag="p2")
                    for h in range(2):
                        _emit_conv(nc, p2[:, h * 512:(h + 1) * 512],
                                   [wdr[f"w2{m}"] for m in range(3)],
                                   c1, RB_S, G + 4 * h, first=True)
                    for rho in range(2):
                        for h in range(2):
                            sl = (G + 4 * h + rho) % RB_OY
                            nc.tensor.matmul(
                                p2[rho * D:(rho + 1) * D,
                                   h * 512:(h + 1) * 512],
                                wavt, oy[0:D, sl:sl + 3:2, :],
                                start=False, stop=(rho == 1 and h == 1),
                                skip_group_check=True)
                    for rho in range(2):
                        sl = (G + rho) % RB_OY
                        wr(oy[D:2 * D, sl:sl + 7:2, :],
                           p2[rho * D:(rho + 1) * D, :],
                           bias['bo3b'][0:D], False, 'av'[rho])
                    # o3f: fp8 row-q plane via gpsimd from o3b
                    slo = G % RB_OY
                    qo = G % RB_S
                    nc.gpsimd.tensor_scalar_add(
                        out=o3f[D:2 * D, qo:qo + 8, 1:257],
                        in0=oy[D:2 * D, slo:slo + 8, :],
                        scalar1=bias['bn4'][D:2 * D])
                    dup_dma(o3f, v, src_lo=False)

                # ---- conv3 (reads o3f) -> c3
                u = w - L3
                if 0 <= u < NW:
                    G = 8 * u
                    p3 = ps3.tile([128, 1024], F32, tag="p3")
                    for h in range(2):
                        _emit_conv(nc, p3[:, h * 512:(h + 1) * 512],
                                   [wdr[f"w3{m}"] for m in range(3)],
                                   o3f, RB_S, G + 4 * h, first=True,
                                   stop_last=(h == 1))
                    wr_stage(c3, p3, G, bias['bc3'][0:D], ('v', 'v'))
                    dup_dma(c3, u, src_lo=True)

                # ---- conv4 (reads c3) + ii-inj -> out
                z = w - L4
                if 0 <= z < NW:
                    G = 8 * z
                    p4 = ps4.tile([128, 1024], F32, tag="p4")
                    for h in range(2):
                        _emit_conv(nc, p4[:, h * 512:(h + 1) * 512],
                                   [wdr[f"w4{m}"] for m in range(3)],
                                   c3, RB_S, G + 4 * h, first=True)
                    for rho in range(2):
                        for h in range(2):
                            sl = (G + 4 * h + rho) % RB_OY
                            nc.tensor.matmul(
                                p4[rho * D:(rho + 1) * D,
                                   h * 512:(h + 1) * 512],
                                ii, oy[:, sl:sl + 3:2, :],
                                start=False, stop=(rho == 1 and h == 1),
                                skip_group_check=True)
                    ot = ots.tile([128, 4, W_IMG], F32, tag="ot")
                    nc.scalar.activation(out=ot, in_=p4, func=AF.Copy,
                                         bias=0.0, scale=1.0)
                    nc.sync.dma_start(out=out_d[:, 4 * z:4 * z + 4, :], in_=ot)
    nc.compile()
    return nc


_NC_CACHE = {}


def _get_ncs():
    if "a" not in _NC_CACHE:
        _NC_CACHE["a"] = _build_nc_a()
        _NC_CACHE["b"] = _build_nc_b()
    return _NC_CACHE["a"], _NC_CACHE["b"]


def _host_fold(G, Sx, Sy, Wq, bq, Wk, bk, Vw, vb):
    """Raw Gram [128,128] + channel sums -> (Wav [64,64], bav [64]) in f64."""
    G = G.astype(np.float64)
    Gxx, Gxy, Gyy = G[:D, :D], G[:D, D:], G[D:, D:]
    n = float(HW)
    QK = (Wq @ Gxy @ Wk.T + np.outer(Wq @ Sx, bk)
          + np.outer(bq, Wk @ Sy) + n * np.outer(bq, bk))
    qq = np.einsum('ij,jk,ik->i', Wq, Gxx, Wq) + 2 * bq * (Wq @ Sx) + n * bq * bq
    kk = np.einsum('ij,jk,ik->i', Wk, Gyy, Wk) + 2 * bk * (Wk @ Sy) + n * bk * bk
    St = QK / np.maximum(np.sqrt(qq), EPS)[:, None] \
            / np.maximum(np.sqrt(kk), EPS)[None, :]
    A = np.zeros((D, D))
    for h in range(4):
        blk = St[16 * h:16 * h + 16, 16 * h:16 * h + 16]
        e = np.exp(blk - blk.max(axis=1, keepdims=True))
        A[16 * h:16 * h + 16, 16 * h:16 * h + 16] = e / e.sum(axis=1, keepdims=True)
    return A @ Vw, A @ vb


def _taps(w):
    """w [o, i, 3, 3] -> Wtap[ky+1][kx+1] = [o, i] f64."""
    w = w.astype(np.float64)
    return [[w[:, :, ky, kx] for kx in range(3)] for ky in range(3)]


def kernel(x, y, qw, qb, kw, kb, vw, vb,
           r1w1, r1b1, r1w2, r1b2, r2w1, r2b1, r2w2, r2b2, **_):
    x = np.asarray(x, np.float32)
    y = np.asarray(y, np.float32)
    qw, qb, kw, kb = (np.asarray(a, np.float32) for a in (qw, qb, kw, kb))
    vw, vb = np.asarray(vw, np.float32), np.asarray(vb, np.float32)
    r1w1, r1b1, r1w2, r1b2 = (np.asarray(a, np.float32) for a in (r1w1, r1b1, r1w2, r1b2))
    r2w1, r2b1, r2w2, r2b2 = (np.asarray(a, np.float32) for a in (r2w1, r2b1, r2w2, r2b2))
    nca, ncb = _get_ncs()

    # ---- pass A: fp8 pixel-major Gram
    in_maps_a = []
    xs_l, ys_l = [], []
    for c in range(NCORES):
        xq = x[c].reshape(D, HW).astype(E4)
        yq = y[c].reshape(D, HW).astype(E4)
        xs_l.append(xq.astype(np.float64).sum(axis=1))
        ys_l.append(yq.astype(np.float64).sum(axis=1))
        Z = np.empty((HW, 2 * D), E4)
        Z[:, :D] = xq.T
        Z[:, D:] = yq.T
        # za[t, p, 256j + 128u + c] = Z[4096 t + 256 j + 128 u + p, c]
        za = Z.reshape(16, 16, 2, 128, 128).transpose(0, 3, 1, 2, 4) \
              .reshape(16, 128, 4096)
        in_maps_a.append({"za": np.ascontiguousarray(za)})
    res_a = run_bass_kernel_spmd(nca, in_maps_a, core_ids=list(range(NCORES)))

    # ---- host fold + pass-B inputs
    Wq, Wk, Vw = (qw[:, :, 0, 0].astype(np.float64),
                  kw[:, :, 0, 0].astype(np.float64),
                  vw[:, :, 0, 0].astype(np.float64))
    bq64, bk64, vb64 = (qb.astype(np.float64), kb.astype(np.float64),
                        vb.astype(np.float64))
    t1, t2, t3, t4 = _taps(r1w1), _taps(r1w2), _taps(r2w1), _taps(r2w2)
    w_shared = {}
    for ci, tp in ((2, t2), (3, t3), (4, t4)):
        for m, arr in enumerate(_conv_w3(tp)):
            w_shared[f"w{ci}{m}"] = arr
    ii = np.concatenate([np.eye(D, dtype=np.float32)] * 2, axis=0).astype(BF)

    in_maps_b = []
    for c in range(NCORES):
        Wav, bav = _host_fold(res_a.results[c]["gout"], xs_l[c], ys_l[c],
                              Wq, bq64, Wk, bk64, Vw, vb64)
        # conv1 fold: W1' = W1 @ Wav; bias + exact boundary corrections
        t1p = [[4.0 * (t1[ky][kx] @ Wav) for kx in range(3)] for ky in range(3)]
        b1i = r1b1.astype(np.float64) + sum(t1[ky][kx] @ bav
                                            for ky in range(3) for kx in range(3))
        corr = np.zeros((1, 10 * D), np.float64)  # conv1-psum units (x4)
        cl = -sum(t1[ky][0] @ bav for ky in range(3))
        cr = -sum(t1[ky][2] @ bav for ky in range(3))
        corr[0, 0:D] = cl
        corr[0, D:2 * D] = cl
        corr[0, 2 * D:3 * D] = cr
        corr[0, 3 * D:4 * D] = cr
        corr[0, 4 * D:5 * D] = -sum(t1[0][kx] @ bav for kx in range(3))  # top
        corr[0, 5 * D:6 * D] = -sum(t1[2][kx] @ bav for kx in range(3))  # bot
        corr[0, 6 * D:7 * D] = t1[0][0] @ bav   # corner (0,0)
        corr[0, 7 * D:8 * D] = t1[0][2] @ bav   # corner (0,255)
        corr[0, 8 * D:9 * D] = t1[2][0] @ bav   # corner (255,0)
        corr[0, 9 * D:10 * D] = t1[2][2] @ bav  # corner (255,255)

        corr *= 4.0

        # yf8d [128, 264, 258]: dram slot t holds image row t-1 (p 0:64)
        # and image row t (p 64:128); zeros pad the borders.
        y8 = np.zeros((128, 264, RS), E4)
        ycf8 = y[c].reshape(D, H, W_IMG).astype(E4)
        y8[0:D, 1:257, 1:257] = ycf8
        y8[D:2 * D, 0:256, 1:257] = ycf8

        m = {"yb": np.ascontiguousarray(y[c].reshape(D, H, W_IMG).astype(BF)),
             "y8": np.ascontiguousarray(y8),
             "wavt": np.ascontiguousarray(Wav.T.astype(np.float32).astype(BF)),
             "ii": ii,
             "corr": corr.astype(np.float32).astype(BF),
             "bc1": np.tile(b1i.astype(np.float32).reshape(D, 1), (2, 1)),
             "bo3b": np.tile((bav + r1b2.astype(np.float64)
                              + r2b2.astype(np.float64))
                             .astype(np.float32).reshape(D, 1), (2, 1)),
             "bn4": np.tile((-r2b2).astype(np.float32).reshape(D, 1), (2, 1)),
             "bc3": np.tile(r2b1.reshape(D, 1), (2, 1))}
        for mm, arr in enumerate(_conv_w3(t1p)):
            m[f"w1{mm}"] = arr
        m.update(w_shared)
        in_maps_b.append({k: np.ascontiguousarray(v) for k, v in m.items()})
    res_b = run_bass_kernel_spmd(ncb, in_maps_b, core_ids=list(range(NCORES)))

    out = np.empty((NCORES, D, H, W_IMG), np.float32)
    for c in range(NCORES):
        buf = res_b.results[c]["out"].reshape(2, D, 128, W_IMG)
        out[c, :, 0::2, :] = buf[0]
        out[c, :, 1::2, :] = buf[1]
    return out


if __name__ == "__main__":
    rng = np.random.default_rng(0)
    ins = {
        "x": rng.standard_normal((8, D, H, W_IMG)).astype(np.float32),
        "y": rng.standard_normal((8, D, H, W_IMG)).astype(np.float32),
        "qw": (rng.standard_normal((D, D, 1, 1)) / 8).astype(np.float32),
        "qb": (rng.standard_normal(D) / 8).astype(np.float32),
        "kw": (rng.standard_normal((D, D, 1, 1)) / 8).astype(np.float32),
        "kb": (rng.standard_normal(D) / 8).astype(np.float32),
        "vw": (rng.standard_normal((D, D, 1, 1)) / 8).astype(np.float32),
        "vb": (rng.standard_normal(D) / 8).astype(np.float32),
    }
    for i in (1, 2):
        for j in (1, 2):
            ins[f"r{i}w{j}"] = (rng.standard_normal((D, D, 3, 3)) / 24).astype(np.float32)
            ins[f"r{i}b{j}"] = (rng.standard_normal(D) / 24).astype(np.float32)
    o = kernel(**ins)
    print("kernel ran, out shape", o.shape, "std", o.std())
